# revision 11
# baseline (speedup 1.0000x reference)
"""Fused 3/2 polyphase resampler + 51-tap FIR on 8 Trainium2 NeuronCores.

Math: reference = lfilter(b, 1, resample_poly(x, 3, 2, h)) (plus padding
details). Both FIRs fuse into one 163-tap filter g = h * upsample2(b), and
out[t] = sum_n x[n] * g[2t + 32 - 3n]. A block of 300 consecutive outputs is
X_win[256, .]^T @ M[256, 300] with banded M[pp, j] = g[2j + 161 - 3pp] and
X_win the 256-sample input window starting at 2*(t0/3) - 43.

The host performs a 1.33x-amplified im2col into bf16 [256, cols] arrays
(column order chosen so PSUM tiles concatenate into flat-contiguous output),
shards outputs across 8 cores, and fixes the 50 head / 96 tail outputs where
the fused filter is invalid (the intermediate resampler output is clipped
there). The device does, per 128-column tile: 2 accumulating bf16 matmuls
(contraction 256 split 128+128), a PSUM->SBUF copy, and a contiguous DMA out.
"""

import numpy as np
import ml_dtypes

import concourse.bass as bass
import concourse.bacc as bacc_mod
import concourse.mybir as mybir
import concourse.tile as tile
from concourse.bass_utils import run_bass_kernel_spmd

UP, DOWN = 3, 2
B, N = 8, 1048576
N_IN = B * N                      # 8388608
N_OUT = N_IN * UP // DOWN         # 12582912
PER_B = (N_OUT + 100) // B        # 1572876
TOTAL = B * PER_B                 # 12583008
LG = 163

F = 165
BLK = 3 * F                       # 495 outputs per block
WIN = 384                         # 2F + 54, split into 3x128 contraction
S_CORE = PER_B // 3               # 524292
NTILES = 25
NBLK = NTILES * 128               # 3200 blocks per core (padded)
NCOLS = NTILES * 128              # 3200
GROUPS = [(t, 4) for t in range(0, 24, 4)] + [(24, 1)]
XCHUNKS = [(0, 4), (4, 8), (12, 8), (20, 5)]   # (tile_start, n_tiles)
# nonzero bands of the three 128-row chunks of M (verified numerically)
J1 = (0, 192)
J2 = (112, 384)
J3 = (304, 495)
PAD_L, PAD_R = 43, 8192

BF16 = mybir.dt.bfloat16
F32 = mybir.dt.float32
F16 = mybir.dt.float16

_cache = {}


def _build_filters(h, b):
    h = np.asarray(h, np.float64)
    b = np.asarray(b, np.float64)
    g = np.zeros(LG)
    for m in range(b.shape[0]):
        g[2 * m: 2 * m + h.shape[0]] += b[m] * h
    return g


def _build_M(g):
    pp = np.arange(WIN)[:, None]
    j = np.arange(BLK)[None, :]
    idx = 2 * j + 161 - 3 * pp
    ok = (idx >= 0) & (idx < LG)
    return np.where(ok, g[np.clip(idx, 0, LG - 1)], 0.0)


def _block_to_col():
    col_of_block = np.empty(NBLK, np.int64)
    for gi, (tb, G) in enumerate(GROUPS):
        for j in range(G):
            t = tb + j
            q = np.arange(128)
            col_of_block[tb * 128 + q * G + j] = t * 128 + q
    return col_of_block


_COL_OF_BLOCK = _block_to_col()


def _prep_core_X(xpad_bf16, core):
    starts = np.empty(NCOLS, np.int64)
    s_q = S_CORE * core + np.arange(NBLK) * F
    starts[_COL_OF_BLOCK] = 2 * s_q
    sw = np.lib.stride_tricks.sliding_window_view(xpad_bf16, WIN)
    return np.ascontiguousarray(sw[starts].T)     # [256, NCOLS] bf16


def _u_window(x64, h64, i_arr):
    out = np.zeros(len(i_arr))
    for k, i in enumerate(i_arr):
        lo = max(0, -(-(2 * i - 30) // 3))
        hi = min(N_IN - 1, (2 * i + 32) // 3)
        n = np.arange(lo, hi + 1)
        out[k] = np.dot(x64[n], h64[2 * i + 32 - 3 * n])
    return out


def _head_tail_fix(x64, h64, b64):
    u_head = _u_window(x64, h64, np.arange(0, 50))
    head = np.zeros(50)
    for t in range(50):
        m = np.arange(0, t + 1)
        head[t] = np.dot(b64[m], u_head[t - m])
    u_tail = _u_window(x64, h64, np.arange(N_OUT - 50, N_OUT))
    tail = np.zeros(96)
    for k in range(96):
        t = N_OUT + k
        m = np.arange(k + 1, 51)
        i = t - m - (N_OUT - 50)
        ok = i >= 0
        tail[k] = np.dot(b64[m[ok]], u_tail[i[ok]])
    return head, tail


def _build_nc():
    nc = bacc_mod.Bacc(None)
    X1 = nc.dram_tensor("x1", [128, NCOLS], BF16, kind="ExternalInput")
    X2 = nc.dram_tensor("x2", [128, NCOLS], BF16, kind="ExternalInput")
    X3 = nc.dram_tensor("x3", [128, NCOLS], BF16, kind="ExternalInput")
    M1 = nc.dram_tensor("m1", [128, BLK], BF16, kind="ExternalInput")
    M2 = nc.dram_tensor("m2", [128, BLK], BF16, kind="ExternalInput")
    M3 = nc.dram_tensor("m3", [128, BLK], BF16, kind="ExternalInput")
    OUT = nc.dram_tensor("out", [NBLK * BLK], F16, kind="ExternalOutput")

    with tile.TileContext(nc) as tc:
        with (
            tc.tile_pool(name="const", bufs=1) as constp,
            tc.tile_pool(name="stage", bufs=4) as sp,
            tc.tile_pool(name="psum", bufs=8, space="PSUM") as pp,
        ):
            m1 = constp.tile([128, BLK], BF16)
            m2 = constp.tile([128, BLK], BF16)
            m3 = constp.tile([128, BLK], BF16)
            nc.sync.dma_start(m1[:], M1[:])
            nc.scalar.dma_start(m2[:], M2[:])
            nc.gpsimd.dma_start(m3[:], M3[:])
            # X resident in SBUF, loaded in chunks so compute starts early;
            # x1/x3 via SWDGE (Pool) and x2 via ACT-HWDGE so issue overlaps
            x1c, x2c, x3c = [], [], []
            for cs, cl in XCHUNKS:
                c1 = constp.tile([128, cl * 128], BF16, tag=f"x1c{cs}")
                c2 = constp.tile([128, cl * 128], BF16, tag=f"x2c{cs}")
                c3 = constp.tile([128, cl * 128], BF16, tag=f"x3c{cs}")
                nc.gpsimd.dma_start(c1[:], X1[:, cs * 128:(cs + cl) * 128])
                nc.scalar.dma_start(c2[:], X2[:, cs * 128:(cs + cl) * 128])
                nc.gpsimd.dma_start(c3[:], X3[:, cs * 128:(cs + cl) * 128])
                x1c.append((cs, cl, c1))
                x2c.append((cs, cl, c2))
                x3c.append((cs, cl, c3))

            def xslice(chunks, t):
                for cs, cl, tl in chunks:
                    if cs <= t < cs + cl:
                        o = (t - cs) * 128
                        return tl[:, o:o + 128]
                raise AssertionError(t)

            for gi, (tb, G) in enumerate(GROUPS):
                st = sp.tile([128, G * BLK], F16, tag="st")
                for j in range(G):
                    t = tb + j
                    ps = pp.tile([128, BLK], F32)
                    nc.tensor.matmul(ps[:, J1[0]:J1[1]], xslice(x1c, t),
                                     m1[:, J1[0]:J1[1]], start=True, stop=False)
                    nc.tensor.matmul(ps[:, J2[0]:J2[1]], xslice(x2c, t),
                                     m2[:, J2[0]:J2[1]], start=False, stop=False)
                    nc.tensor.matmul(ps[:, J3[0]:J3[1]], xslice(x3c, t),
                                     m3[:, J3[0]:J3[1]], start=False, stop=True)
                    dst = st[:, j * BLK:(j + 1) * BLK]
                    if j % 2 == 0:
                        nc.vector.tensor_copy(dst, ps[:])
                    else:
                        nc.scalar.copy(dst, ps[:])
                base = tb * 128 * BLK
                dest = OUT[base: base + 128 * G * BLK].rearrange(
                    "(p f) -> p f", p=128)
                nc.sync.dma_start(dest, st[:])
    if not nc.is_finalized():
        nc.finalize()
    return nc


def _prepare(x, h_resample, b_fir):
    xf32 = np.ascontiguousarray(np.asarray(x, np.float32).reshape(-1))
    g = _build_filters(h_resample, b_fir)
    M = _build_M(g).astype(ml_dtypes.bfloat16)
    xpad = np.concatenate([
        np.zeros(PAD_L, np.float32), xf32, np.zeros(PAD_R, np.float32)
    ]).astype(ml_dtypes.bfloat16)
    in_maps = []
    for core in range(8):
        X = _prep_core_X(xpad, core)
        in_maps.append({
            "x1": np.ascontiguousarray(X[:128]),
            "x2": np.ascontiguousarray(X[128:256]),
            "x3": np.ascontiguousarray(X[256:]),
            "m1": np.ascontiguousarray(M[:128]),
            "m2": np.ascontiguousarray(M[128:256]),
            "m3": np.ascontiguousarray(M[256:]),
        })
    return xf32, in_maps


def _postprocess(xf32, h_resample, b_fir, results):
    flat = np.empty(TOTAL, np.float32)
    for core in range(8):
        flat[core * PER_B:(core + 1) * PER_B] = \
            results[core]["out"][:PER_B].astype(np.float32)
    head, tail = _head_tail_fix(
        xf32.astype(np.float64),
        np.asarray(h_resample, np.float64),
        np.asarray(b_fir, np.float64))
    flat[:50] = head
    flat[N_OUT:N_OUT + 96] = tail
    return flat.reshape(B, PER_B)


def run(x, h_resample, b_fir, trace=False, **run_kwargs):
    xf32, in_maps = _prepare(x, h_resample, b_fir)
    if "nc" not in _cache:
        _cache["nc"] = _build_nc()
    last_err = None
    for _attempt in range(3):
        try:
            res = run_bass_kernel_spmd(
                _cache["nc"], in_maps, core_ids=list(range(8)),
                trace=trace, **run_kwargs)
            break
        except Exception as e:  # transient NRT/device hiccups — retry
            last_err = e
    else:
        raise last_err
    out = _postprocess(xf32, h_resample, b_fir, res.results)
    return out, res


def kernel(x, h_resample, b_fir):
    out, _ = run(x, h_resample, b_fir, trace=False)
    return out
